# revision 35
# baseline (speedup 1.0000x reference)
"""Trainium2 Bass kernel for the (non-standard) MultiHeadAttention module.

Reference math (B=4, N=2048, E=512, H=8):
    q/k/v  = x @ W{q,k,v} + b          # (B, N, E*H)
    split:   head h takes columns h::H  -> per-head (N, E) matrices
    attT_h = (k_h^T @ q_h) * 1/sqrt(N) # (f, e) -- attention over the E axis
    A_h    = exp(attT_h)               # softmax numerator (logits reach ~33,
                                       #  exp ~1e14: fits f32/bf16, NOT fp8)
    s_h[e] = sum_f A_h[f, e]
    out row n' = 4e + r gets  sum_hl (A_h^T/s_h) @ P_h + bp
      for h = 2r + hl  (consequence of the reference's raw
      (B,E,H,N)->(B,N,E*H) reshape before the output projection), where
    P_h    = v_h^T @ Wp_half(hl) + bp/2

Key algebraic refactors (this module attends over the E axis and contracts
over n, so everything collapses into E x E space):
  * Gram matrix  X = x_b^T @ x_b  (E x E, once per core):
      attT_h = Wk_h^T X Wq_h + (Wk_h^T xs) (x) bq_h
               + bk_h (x) (Wq_h^T xs + N bq_h),   xs = colsum(x_b)
    -- eliminates the q/k projections entirely.  The two rank-1 updates are
    merged into one contraction-2 matmul (rk1 tensor).
  * (A @ v^T) @ Wp == A @ (v^T @ Wp) and
    v_h^T @ Wp_hl == Wv_h^T @ G_hl + bv_h (x) colsum(Wp_hl)  with
    G_hl = x_b^T @ Wp_hl computed once per core -- eliminates the v
    projection and the big P matmuls.  bv (x) swp and the bp/2 constant are
    one contraction-2 matmul (pb2 tensor).
  * X is symmetric: only the 10 upper-triangle 128x128 blocks are computed
    on the PE; the 6 lower blocks are filled by SBUF->SBUF DMA transposes
    (xbar), which cost no PE time.
  * softmax normalization at the very end:
    out = U0*r0 + U1*r1,  U_h = A_h^T @ P_h,  r_h = 1/s_h; the output is
    stored/DMA'd in bf16 and widened to f32 on the host.

Everything runs in bf16 with f32 PSUM accumulation: fp32/f32r moving
operands stream at half rate on real TRN2 HW, and fp8 fails BOTH ways --
DoubleRow matmuls measure ~600ns (slower than 2 bf16 matmuls) and, with
sharply peaked attention (logits to +-33), out ~= P[argmax f, :], so fp8's
~3-5% error on the P chain lands directly on the output (measured 5.2e-2
vs the 2e-2 budget; bf16 measures ~5e-3).

Sharding: 16 independent units (b, r), b in 0..3, r in 0..3; unit (b, r)
owns heads {2r, 2r+1} and produces output rows out[b, r::4, :].  Two units
per core, batch-major:  core c -> b = c//2, r in {2*(c%2), 2*(c%2)+1}.
No inter-core communication.
"""

import numpy as np
import ml_dtypes
from contextlib import ExitStack

import concourse.bass as bass
import concourse.mybir as mybir
import concourse.tile as tile
from concourse import bacc
from concourse.bass_utils import run_bass_kernel_spmd

B, N, E, H = 4, 2048, 512, 8
NT = N // 128          # 16 contraction chunks of 128 over n
NP2 = NT // 2          # 8 chunk-pairs (xn tiles hold 2 chunks)
EB = E // 128          # 4 blocks of 128 over e/f
SCALE = float(1.0 / np.sqrt(np.float32(N)))
F32 = mybir.dt.float32
BF16 = mybir.dt.bfloat16
NP_BF16 = ml_dtypes.bfloat16
PSUM = bass.MemorySpace.PSUM
EXP = mybir.ActivationFunctionType.Exp
COPY = mybir.ActivationFunctionType.Copy

_CACHED_NC = None


def _rows_ap(dram, row0, nrow, width, nbatch):
    """AP over dram rows [row0, row0+nrow*nbatch) shaped [nrow, nbatch, width]
    (partition-first chunked layout for a [128, nbatch, width] SBUF tile)."""
    return bass.AP(
        tensor=dram,
        offset=row0 * width,
        ap=[[width, nrow], [nrow * width, nbatch], [1, width]],
    )


def build_nc():
    nc = bacc.Bacc("TRN2", target_bir_lowering=False, debug=False)

    xn_d = nc.dram_tensor("xn", (N, E), BF16, kind="ExternalInput")
    wp_d = nc.dram_tensor("wp", (2, N, E), BF16, kind="ExternalInput")
    wq_d = nc.dram_tensor("wq", (2, 2, 128, EB, E), BF16, kind="ExternalInput")
    wk_d = nc.dram_tensor("wk", (2, 2, 128, EB, E), BF16, kind="ExternalInput")
    wv_d = nc.dram_tensor("wv", (2, 2, 128, EB, E), BF16, kind="ExternalInput")
    pb2_d = nc.dram_tensor("pb2", (2, 2, 2, 2, E), BF16, kind="ExternalInput")
    rk1_d = nc.dram_tensor("rk1", (2, 2, 2, 2, E), BF16, kind="ExternalInput")
    out_d = nc.dram_tensor("out", (2, E, E), BF16, kind="ExternalOutput")

    with tile.TileContext(nc) as tc, ExitStack() as ctx:
        consts = ctx.enter_context(tc.tile_pool(name="consts", bufs=1))
        stream = ctx.enter_context(tc.tile_pool(name="stream", bufs=8))
        wqkv_pool = ctx.enter_context(tc.tile_pool(name="wqkv", bufs=2))
        t1_pool = ctx.enter_context(tc.tile_pool(name="t1", bufs=1))
        a_pool = ctx.enter_context(tc.tile_pool(name="a", bufs=2))
        p_pool = ctx.enter_context(tc.tile_pool(name="p", bufs=1))
        o_pool = ctx.enter_context(tc.tile_pool(name="o", bufs=4))
        r_pool = ctx.enter_context(tc.tile_pool(name="r", bufs=4))
        mm_ps = ctx.enter_context(tc.tile_pool(name="mmps", bufs=2, space=PSUM))
        big_ps = ctx.enter_context(tc.tile_pool(name="bigps", bufs=1, space=PSUM))
        u_ps = ctx.enter_context(tc.tile_pool(name="ups", bufs=2, space=PSUM))

        # ---- PE warm-up: dummy matmuls on a memset tile so the HAM clock
        # gate flips to 8/8 before the real work arrives ----
        warm_sb = consts.tile([128, E], BF16, tag="warm")
        nc.vector.memset(warm_sb[:], 0.0)
        warm_ps = mm_ps.tile([128, E], F32, tag="mm", name="warm")
        for i in range(8):
            nc.tensor.matmul(
                warm_ps[:], warm_sb[:, 0:128], warm_sb[:], start=i == 0, stop=i == 7
            )

        # ---- streamed inputs, batched DMAs ----
        # sync: xn (8 x 2-chunk); gpsimd: wp halves (8 x 4-chunk) + per-head
        # wv/wq; scalar: consts + per-head wk.
        xn_sb = []
        for i in range(8):
            t = consts.tile([128, 2, E], BF16, tag=f"xn{i}", name=f"xn{i}")
            nc.sync.dma_start(out=t[:], in_=_rows_ap(xn_d, i * 256, 128, E, 2))
            xn_sb.append(t)
        wp_sb = {}
        wp1_bis = []
        for hl in range(2):
            for i in range(8):
                t = stream.tile([128, 2, E], BF16, tag="wp", name=f"wp_{hl}_{i}", bufs=16)
                bi = nc.gpsimd.dma_start(
                    out=t[:], in_=_rows_ap(wp_d, hl * N + i * 256, 128, E, 2)
                )
                if hl == 1:
                    wp1_bis.append(bi)
                wp_sb[(hl, i)] = t

        # ---- other resident constants (scalar queue) ----
        pb2_sb = consts.tile([2, 2, 2, 2, E], BF16, tag="pb2")
        nc.scalar.dma_start(out=pb2_sb[:], in_=pb2_d.ap())
        rk1_sb = consts.tile([2, 2, 2, 2, E], BF16, tag="rk1")
        nc.scalar.dma_start(out=rk1_sb[:], in_=rk1_d.ap())

        # ---- P tiles (persistent) + all-ones vector for s ----
        P_sbs = [
            p_pool.tile([128, EB, E], BF16, tag=f"p{i}", name=f"P{i}")
            for i in range(2)
        ]
        ones_sb = consts.tile([128, 2], BF16, tag="ones")
        nc.vector.memset(ones_sb[:], 1.0)

        # ---- pass 1: X = x^T x (big arena) + G0 = x^T Wp0 (mm/u slots) ----
        X_ps = big_ps.tile([128, EB, E], F32, tag="big")
        g_sb = [
            consts.tile([128, EB, E], BF16, tag=f"g{hl}", name=f"g{hl}")
            for hl in range(2)
        ]
        g0_slots = [
            mm_ps.tile([128, E], F32, tag="mm", name="g0a"),
            mm_ps.tile([128, E], F32, tag="mm", name="g0b"),
            u_ps.tile([128, E], F32, tag="u", name="g0c"),
            u_ps.tile([128, E], F32, tag="u", name="g0d"),
        ]
        gate_g0 = None
        for np_ in range(NP2):
            xt = xn_sb[np_]
            wpt = wp_sb[(0, np_)]

            def emit_x_np():
                for m in range(EB):
                    msl = slice(m * 128, (m + 1) * 128)
                    for j in range(2):
                        nc.tensor.matmul(
                            X_ps[:, m, m * 128 :],
                            xt[:, j, msl],
                            xt[:, j, m * 128 :],
                            start=np_ == 0 and j == 0,
                            stop=np_ == NP2 - 1 and j == 1,
                        )

            def emit_g0_np():
                nonlocal gate_g0
                for m in range(EB):
                    msl = slice(m * 128, (m + 1) * 128)
                    for j in range(2):
                        g_bi = nc.tensor.matmul(
                            g0_slots[m][:],
                            xt[:, j, msl],
                            wpt[:, j, :],
                            start=np_ == 0 and j == 0,
                            stop=np_ == NP2 - 1 and j == 1,
                        )
                    if np_ == 1 and m == 0:
                        gate_g0 = g_bi.ins

            if np_ < NP2 - 1:
                # X first, then G0 -- gives the wp stream extra slack so its
                # DMA never stalls the PE
                emit_x_np()
                emit_g0_np()
            else:
                # last chunk-pair: G0 first so its accumulation stops ~1us
                # earlier and the g casts (which gate pass-2 psum reuse)
                # overlap the remaining X matmuls
                emit_g0_np()
                emit_x_np()
            if np_ == 1:
                for bi in wp1_bis:
                    tile.add_dep_helper(bi.ins, gate_g0, reason="delay wp1")
        # g casts first (they gate pass-2 psum reuse), X casts after
        nc.vector.tensor_copy(g_sb[0][:, 0, :], g0_slots[0][:])
        nc.scalar.activation(g_sb[0][:, 1, :], g0_slots[1][:], COPY)
        nc.vector.tensor_copy(g_sb[0][:, 2, :], g0_slots[2][:])
        nc.scalar.activation(g_sb[0][:, 3, :], g0_slots[3][:], COPY)
        X_sb = consts.tile([128, EB, E], BF16, tag="X")
        nc.vector.tensor_copy(X_sb[:, 0, :], X_ps[:, 0, :])
        nc.scalar.activation(X_sb[:, 1, 128:], X_ps[:, 1, 128:], COPY)
        nc.vector.tensor_copy(X_sb[:, 2, 256:], X_ps[:, 2, 256:])
        nc.scalar.activation(X_sb[:, 3, 384:], X_ps[:, 3, 384:], COPY)

        # ---- pass 2: G1 = x^T Wp1 ----
        g1_slots = [
            mm_ps.tile([128, E], F32, tag="mm", name="g1a"),
            mm_ps.tile([128, E], F32, tag="mm", name="g1b"),
            u_ps.tile([128, E], F32, tag="u", name="g1c"),
            u_ps.tile([128, E], F32, tag="u", name="g1d"),
        ]
        gate_g1 = None
        for np_ in range(NP2):
            xt = xn_sb[np_]
            wpt = wp_sb[(1, np_)]
            for m in range(EB):
                msl = slice(m * 128, (m + 1) * 128)
                for j in range(2):
                    g_bi = nc.tensor.matmul(
                        g1_slots[m][:],
                        xt[:, j, msl],
                        wpt[:, j, :],
                        start=np_ == 0 and j == 0,
                        stop=np_ == NP2 - 1 and j == 1,
                    )
                if np_ == 1 and m == 0:
                    gate_g1 = g_bi.ins
            if np_ == 2:
                # X is symmetric: fill the lower-triangle blocks with
                # SBUF->SBUF DMA transposes of the upper blocks (no PE cost)
                engs = [nc.sync, nc.scalar]
                k = 0
                for mt in range(1, EB):
                    for ct in range(mt):
                        engs[k % 2].dma_start_transpose(
                            out=X_sb[:, mt, ct * 128 : (ct + 1) * 128],
                            in_=X_sb[:, ct, mt * 128 : (mt + 1) * 128],
                        )
                        k += 1
        nc.vector.tensor_copy(g_sb[1][:, 0, :], g1_slots[0][:])
        nc.scalar.activation(g_sb[1][:, 1, :], g1_slots[1][:], COPY)
        nc.vector.tensor_copy(g_sb[1][:, 2, :], g1_slots[2][:])
        nc.scalar.activation(g_sb[1][:, 3, :], g1_slots[3][:], COPY)

        gate_hist = [gate_g0, gate_g1]  # per-head early gates
        pending = {"srow": None}

        def pump_s():
            if pending["srow"] is not None:
                A_sb, R_list = pending["srow"]
                pending["srow"] = None
                s_ps = mm_ps.tile([128, EB, 2], F32, tag="mm")
                for eb in range(EB):
                    esl = slice(eb * 128, (eb + 1) * 128)
                    for fc in range(EB):
                        nc.tensor.matmul(
                            s_ps[:, eb, :],
                            A_sb[:, fc, esl],
                            ones_sb[:],
                            start=fc == 0,
                            stop=fc == EB - 1,
                        )
                r_sb = r_pool.tile([128, EB, 2], F32, tag="r")
                nc.vector.reciprocal(out=r_sb[:], in_=s_ps[:])
                R_list.append(r_sb)

        for u in range(2):
            A_tiles, R_tiles = [], []
            for hl in range(2):
                # --- weights for head (u, hl), prefetch-gated ---
                wv_sb = wqkv_pool.tile([128, EB, E], BF16, tag="wv")
                wv_bi = nc.gpsimd.dma_start(out=wv_sb[:], in_=wv_d.ap()[u, hl])
                wq_sb = wqkv_pool.tile([128, EB, E], BF16, tag="wq")
                wq_bi = nc.gpsimd.dma_start(out=wq_sb[:], in_=wq_d.ap()[u, hl])
                wk_sb = wqkv_pool.tile([128, EB, E], BF16, tag="wk")
                wk_bi = nc.sync.dma_start(out=wk_sb[:], in_=wk_d.ap()[u, hl])
                gate = gate_hist[-2]  # two head-phases back
                for bi in (wv_bi, wq_bi, wk_bi):
                    tile.add_dep_helper(bi.ins, gate, reason="delay prefetch")

                P_sb = P_sbs[hl]

                # --- P_h = Wv_h^T @ G_hl + bv (x) swp + bp/2 ---
                def emit_p_group(fb, use_act):
                    fsl = slice(fb * 128, (fb + 1) * 128)
                    p_ps = (u_ps if fb % 2 == 0 else mm_ps).tile(
                        [128, E], F32, tag="u" if fb % 2 == 0 else "mm",
                        name=f"pp{fb}",
                    )
                    first = None
                    for ec in range(EB):
                        bi = nc.tensor.matmul(
                            p_ps[:],
                            wv_sb[:, ec, fsl],
                            g_sb[hl][:, ec, :],
                            start=ec == 0,
                            stop=False,
                        )
                        first = first or bi
                    nc.tensor.matmul(
                        p_ps[:],
                        pb2_sb[0:2, u, hl, 0, fsl],
                        pb2_sb[0:2, u, hl, 1, :],
                        start=False,
                        stop=True,
                    )
                    if use_act:
                        nc.scalar.activation(P_sb[:, fb, :], p_ps[:], COPY)
                    else:
                        nc.vector.tensor_copy(P_sb[:, fb, :], p_ps[:])
                    return first

                def emit_t1(vector_only):
                    T1_ps = big_ps.tile([128, EB, E], F32, tag="big")
                    for m in range(EB):
                        msl = slice(m * 128, (m + 1) * 128)
                        for ec in range(EB):
                            nc.tensor.matmul(
                                T1_ps[:, m, :],
                                X_sb[:, ec, msl],
                                wq_sb[:, ec, :],
                                start=ec == 0,
                                stop=ec == EB - 1,
                            )
                    T1_sb = t1_pool.tile([128, EB, E], BF16, tag="t1")
                    for m in range(EB):
                        if vector_only or m % 2 == 0:
                            nc.vector.tensor_copy(T1_sb[:, m, :], T1_ps[:, m, :])
                        else:
                            nc.scalar.activation(T1_sb[:, m, :], T1_ps[:, m, :], COPY)
                    return T1_sb

                if u == 0 and hl == 0:
                    # first head: nothing to cover before T1, and P's psum
                    # slots want the G1 casts done -- T1's matmuls cover that
                    T1_sb = emit_t1(False)
                    p_first = emit_p_group(0, False)
                    emit_p_group(1, True)
                    emit_p_group(2, True)
                    emit_p_group(3, False)
                elif hl == 0:
                    # first half of P covers the previous head's exp wait
                    p_first = emit_p_group(0, False)
                    emit_p_group(1, True)
                    T1_sb = emit_t1(False)
                    # second half of P covers the T1 casts
                    emit_p_group(2, True)
                    emit_p_group(3, False)
                else:
                    # second head: T1's arena waits for the first head's exp
                    # to drain, so put three P groups ahead of it; all casts
                    # go to Vector so Scalar starts exp with zero queue delay
                    p_first = emit_p_group(0, False)
                    emit_p_group(1, False)
                    emit_p_group(2, False)
                    T1_sb = emit_t1(True)
                    emit_p_group(3, False)
                gate_hist.append(p_first.ins)
                pump_s()

                # --- attT = Wk_h^T @ T1 + rk1 rank-2 update; each block's
                # exp (softmax numerator) chases its accumulation stop ---
                attT_ps = big_ps.tile([128, EB, E], F32, tag="big")
                A_sb = a_pool.tile([128, EB, E], BF16, tag="a")
                for fb in range(EB):
                    fsl = slice(fb * 128, (fb + 1) * 128)
                    for ec in range(EB):
                        nc.tensor.matmul(
                            attT_ps[:, fb, :],
                            wk_sb[:, ec, fsl],
                            T1_sb[:, ec, :],
                            start=ec == 0,
                            stop=False,
                        )
                    nc.tensor.matmul(
                        attT_ps[:, fb, :],
                        rk1_sb[0:2, u, hl, 0, fsl],
                        rk1_sb[0:2, u, hl, 1, :],
                        start=False,
                        stop=True,
                    )
                    nc.scalar.activation(
                        out=A_sb[:, fb, :],
                        in_=attT_ps[:, fb, :],
                        func=EXP,
                        scale=SCALE,
                    )
                A_tiles.append(A_sb)
                pending["srow"] = (A_sb, R_tiles)

            # --- U_h = A_h^T @ P_h ; out = U0*r0 + U1*r1 ---
            out_tiles = [
                o_pool.tile([128, E], BF16, tag="o", name=f"ot{u}_{i}")
                for i in range(EB)
            ]
            for hl in range(2):
                if hl == 1:
                    pump_s()  # s of this unit's second head
                for eb in range(EB):
                    esl = slice(eb * 128, (eb + 1) * 128)
                    u_tile = (u_ps if eb % 2 == 0 else mm_ps).tile(
                        [128, E], F32, tag="u" if eb % 2 == 0 else "mm",
                        name=f"ut{hl}_{eb}",
                    )
                    for fc in range(EB):
                        nc.tensor.matmul(
                            u_tile[:],
                            A_tiles[hl][:, fc, esl],
                            P_sbs[hl][:, fc, :],
                            start=fc == 0,
                            stop=fc == EB - 1,
                        )
                    if hl == 0:
                        # per-partition scaled copy on ACT: frees Vector and
                        # releases the U psum WAR from a second engine
                        nc.scalar.activation(
                            out_tiles[eb][:],
                            u_tile[:],
                            COPY,
                            scale=R_tiles[0][:, eb, 0:1],
                        )
                    else:
                        nc.vector.scalar_tensor_tensor(
                            out_tiles[eb][:],
                            u_tile[:],
                            R_tiles[1][:, eb, 0:1],
                            out_tiles[eb][:],
                            op0=mybir.AluOpType.mult,
                            op1=mybir.AluOpType.add,
                        )
                        (nc.sync if eb % 2 == 0 else nc.scalar).dma_start(
                            out=out_d.ap()[u, eb * 128 : (eb + 1) * 128, :],
                            in_=out_tiles[eb][:],
                        )

    nc.compile()
    return nc


def _get_nc():
    global _CACHED_NC
    if _CACHED_NC is None:
        _CACHED_NC = build_nc()
    return _CACHED_NC


def make_in_maps(x, Wq, bq, Wk, bk, Wv, bv, Wp, bp):
    x = np.asarray(x, np.float32)
    Wq, Wk, Wv, Wp = (np.asarray(a, np.float32) for a in (Wq, Wk, Wv, Wp))
    bq, bk, bv, bp = (np.asarray(a, np.float32) for a in (bq, bk, bv, bp))

    def b16(a):
        return np.ascontiguousarray(a.astype(NP_BF16))

    wp_arr = b16(np.stack([Wp[:N], Wp[N:]]))
    swp = np.stack([Wp[:N].sum(0), Wp[N:].sum(0)])  # (2, E)
    in_maps = []
    for c in range(8):
        b = c // 2
        rs = [2 * (c % 2), 2 * (c % 2) + 1]
        heads = [[2 * r + hl for hl in range(2)] for r in rs]
        xs = x[b].sum(0)

        def tile_w(Wm, h):
            # (E, E) -> [p, t, e] with row t*128+p on partition p
            return Wm[:, h::H].reshape(EB, 128, E).transpose(1, 0, 2)

        wq_arr = b16(np.stack([[tile_w(Wq, h) for h in hu] for hu in heads]))
        wk_arr = b16(np.stack([[tile_w(Wk, h) for h in hu] for hu in heads]))
        wv_arr = b16(np.stack([[tile_w(Wv, h) for h in hu] for hu in heads]))
        # pb2[part, u, hl, role, :]: rank-2 P bias update:
        #   part 0: bv (lhsT) / swp_hl (rhs);  part 1: ones (lhsT) / bp/2
        pb2 = np.empty((2, 2, 2, 2, E), np.float32)
        # rk1[part, u, hl, role, :]: merged rank-2 bias update of attT:
        #   part 0: (Wk^T xs) (x) bq ; part 1: bk (x) (Wq^T xs + N bq)
        rk1 = np.empty((2, 2, 2, 2, E), np.float32)
        for iu, hu in enumerate(heads):
            for ihl, h in enumerate(hu):
                pb2[0, iu, ihl, 0] = bv[h::H]
                pb2[0, iu, ihl, 1] = swp[ihl]
                pb2[1, iu, ihl, 0] = 1.0
                pb2[1, iu, ihl, 1] = 0.5 * bp
                rk1[0, iu, ihl, 0] = Wk[:, h::H].T @ xs
                rk1[0, iu, ihl, 1] = bq[h::H]
                rk1[1, iu, ihl, 0] = bk[h::H]
                rk1[1, iu, ihl, 1] = Wq[:, h::H].T @ xs + np.float32(N) * bq[h::H]
        in_maps.append(
            {
                "xn": b16(x[b]),
                "wp": wp_arr,
                "wq": wq_arr,
                "wk": wk_arr,
                "wv": wv_arr,
                "pb2": b16(pb2),
                "rk1": b16(rk1),
            }
        )
    return in_maps


def assemble_out(results):
    out = np.empty((B, N, E), np.float32)
    for c in range(8):
        b = c // 2
        for ui in range(2):
            r = 2 * (c % 2) + ui
            out[b, r::4, :] = np.asarray(results[c]["out"][ui], np.float32)
    return out


def run(inputs, trace=False, **spmd_kwargs):
    """Full pipeline; returns (output, BassKernelResults)."""
    nc = _get_nc()
    in_maps = make_in_maps(**inputs)
    res = run_bass_kernel_spmd(
        nc, in_maps, core_ids=list(range(8)), trace=trace, **spmd_kwargs
    )
    return assemble_out(res.results), res


def kernel(**inputs):
    out, _ = run(inputs)
    return out


# revision 36
# speedup vs baseline: 1.0028x; 1.0028x over previous
"""Trainium2 Bass kernel for the (non-standard) MultiHeadAttention module.

Reference math (B=4, N=2048, E=512, H=8):
    q/k/v  = x @ W{q,k,v} + b          # (B, N, E*H)
    split:   head h takes columns h::H  -> per-head (N, E) matrices
    attT_h = (k_h^T @ q_h) * 1/sqrt(N) # (f, e) -- attention over the E axis
    A_h    = exp(attT_h)               # softmax numerator (logits reach ~33,
                                       #  exp ~1e14: fits f32/bf16, NOT fp8)
    s_h[e] = sum_f A_h[f, e]
    out row n' = 4e + r gets  sum_hl (A_h^T/s_h) @ P_h + bp
      for h = 2r + hl  (consequence of the reference's raw
      (B,E,H,N)->(B,N,E*H) reshape before the output projection), where
    P_h    = v_h^T @ Wp_half(hl) + bp/2

Key algebraic refactors (this module attends over the E axis and contracts
over n, so everything collapses into E x E space):
  * Gram matrix  X = x_b^T @ x_b  (E x E, once per core):
      attT_h = Wk_h^T X Wq_h + (Wk_h^T xs) (x) bq_h
               + bk_h (x) (Wq_h^T xs + N bq_h),   xs = colsum(x_b)
    -- eliminates the q/k projections entirely.  The two rank-1 updates are
    merged into one contraction-2 matmul (rk1 tensor).
  * (A @ v^T) @ Wp == A @ (v^T @ Wp) and
    v_h^T @ Wp_hl == Wv_h^T @ G_hl + bv_h (x) colsum(Wp_hl)  with
    G_hl = x_b^T @ Wp_hl computed once per core -- eliminates the v
    projection and the big P matmuls.  bv (x) swp and the bp/2 constant are
    one contraction-2 matmul (pb2 tensor).
  * X is symmetric: only the 10 upper-triangle 128x128 blocks are computed
    on the PE; the 6 lower blocks are filled by SBUF->SBUF DMA transposes
    (xbar), which cost no PE time.
  * softmax normalization at the very end:
    out = U0*r0 + U1*r1,  U_h = A_h^T @ P_h,  r_h = 1/s_h; the output is
    stored/DMA'd in bf16 and widened to f32 on the host.

Everything runs in bf16 with f32 PSUM accumulation: fp32/f32r moving
operands stream at half rate on real TRN2 HW, and fp8 fails BOTH ways --
DoubleRow matmuls measure ~600ns (slower than 2 bf16 matmuls) and, with
sharply peaked attention (logits to +-33), out ~= P[argmax f, :], so fp8's
~3-5% error on the P chain lands directly on the output (measured 5.2e-2
vs the 2e-2 budget; bf16 measures ~5e-3).

Sharding: 16 independent units (b, r), b in 0..3, r in 0..3; unit (b, r)
owns heads {2r, 2r+1} and produces output rows out[b, r::4, :].  Two units
per core, batch-major:  core c -> b = c//2, r in {2*(c%2), 2*(c%2)+1}.
No inter-core communication.
"""

import numpy as np
import ml_dtypes
from contextlib import ExitStack

import concourse.bass as bass
import concourse.mybir as mybir
import concourse.tile as tile
from concourse import bacc
from concourse.bass_utils import run_bass_kernel_spmd

B, N, E, H = 4, 2048, 512, 8
NT = N // 128          # 16 contraction chunks of 128 over n
NP2 = NT // 2          # 8 chunk-pairs (xn tiles hold 2 chunks)
EB = E // 128          # 4 blocks of 128 over e/f
SCALE = float(1.0 / np.sqrt(np.float32(N)))
F32 = mybir.dt.float32
BF16 = mybir.dt.bfloat16
NP_BF16 = ml_dtypes.bfloat16
PSUM = bass.MemorySpace.PSUM
EXP = mybir.ActivationFunctionType.Exp
COPY = mybir.ActivationFunctionType.Copy

_CACHED_NC = None


def _rows_ap(dram, row0, nrow, width, nbatch):
    """AP over dram rows [row0, row0+nrow*nbatch) shaped [nrow, nbatch, width]
    (partition-first chunked layout for a [128, nbatch, width] SBUF tile)."""
    return bass.AP(
        tensor=dram,
        offset=row0 * width,
        ap=[[width, nrow], [nrow * width, nbatch], [1, width]],
    )


def build_nc():
    nc = bacc.Bacc("TRN2", target_bir_lowering=False, debug=False)

    xn_d = nc.dram_tensor("xn", (N, E), BF16, kind="ExternalInput")
    wp_d = nc.dram_tensor("wp", (2, N, E), BF16, kind="ExternalInput")
    wq_d = nc.dram_tensor("wq", (2, 2, 128, EB, E), BF16, kind="ExternalInput")
    wk_d = nc.dram_tensor("wk", (2, 2, 128, EB, E), BF16, kind="ExternalInput")
    wv_d = nc.dram_tensor("wv", (2, 2, 128, EB, E), BF16, kind="ExternalInput")
    pb2_d = nc.dram_tensor("pb2", (2, 2, 2, 2, E), BF16, kind="ExternalInput")
    rk1_d = nc.dram_tensor("rk1", (2, 2, 2, 2, E), BF16, kind="ExternalInput")
    out_d = nc.dram_tensor("out", (2, E, E), BF16, kind="ExternalOutput")

    with tile.TileContext(nc) as tc, ExitStack() as ctx:
        consts = ctx.enter_context(tc.tile_pool(name="consts", bufs=1))
        stream = ctx.enter_context(tc.tile_pool(name="stream", bufs=8))
        wqkv_pool = ctx.enter_context(tc.tile_pool(name="wqkv", bufs=2))
        t1_pool = ctx.enter_context(tc.tile_pool(name="t1", bufs=1))
        a_pool = ctx.enter_context(tc.tile_pool(name="a", bufs=2))
        p_pool = ctx.enter_context(tc.tile_pool(name="p", bufs=1))
        o_pool = ctx.enter_context(tc.tile_pool(name="o", bufs=4))
        r_pool = ctx.enter_context(tc.tile_pool(name="r", bufs=4))
        mm_ps = ctx.enter_context(tc.tile_pool(name="mmps", bufs=2, space=PSUM))
        big_ps = ctx.enter_context(tc.tile_pool(name="bigps", bufs=1, space=PSUM))
        u_ps = ctx.enter_context(tc.tile_pool(name="ups", bufs=2, space=PSUM))

        # ---- PE warm-up: dummy matmuls on a memset tile so the HAM clock
        # gate flips to 8/8 before the real work arrives ----
        warm_sb = consts.tile([128, E], BF16, tag="warm")
        nc.vector.memset(warm_sb[:], 0.0)
        warm_ps = mm_ps.tile([128, E], F32, tag="mm", name="warm")
        for i in range(8):
            nc.tensor.matmul(
                warm_ps[:], warm_sb[:, 0:128], warm_sb[:], start=i == 0, stop=i == 7
            )

        # ---- streamed inputs, batched DMAs ----
        # sync: xn (8 x 2-chunk); gpsimd: wp halves (8 x 4-chunk) + per-head
        # wv/wq; scalar: consts + per-head wk.
        xn_sb = []
        for i in range(8):
            t = consts.tile([128, 2, E], BF16, tag=f"xn{i}", name=f"xn{i}")
            nc.sync.dma_start(out=t[:], in_=_rows_ap(xn_d, i * 256, 128, E, 2))
            xn_sb.append(t)
        wp_sb = {}
        wp1_bis = []
        for hl in range(2):
            for i in range(8):
                t = stream.tile([128, 2, E], BF16, tag="wp", name=f"wp_{hl}_{i}", bufs=16)
                bi = nc.gpsimd.dma_start(
                    out=t[:], in_=_rows_ap(wp_d, hl * N + i * 256, 128, E, 2)
                )
                if hl == 1:
                    wp1_bis.append(bi)
                wp_sb[(hl, i)] = t

        # ---- other resident constants (scalar queue) ----
        pb2_sb = consts.tile([2, 2, 2, 2, E], BF16, tag="pb2")
        nc.scalar.dma_start(out=pb2_sb[:], in_=pb2_d.ap())
        rk1_sb = consts.tile([2, 2, 2, 2, E], BF16, tag="rk1")
        nc.scalar.dma_start(out=rk1_sb[:], in_=rk1_d.ap())

        # ---- P tiles (persistent) + all-ones vector for s ----
        P_sbs = [
            p_pool.tile([128, EB, E], BF16, tag=f"p{i}", name=f"P{i}")
            for i in range(2)
        ]
        ones_sb = consts.tile([128, 2], BF16, tag="ones")
        nc.vector.memset(ones_sb[:], 1.0)

        # ---- pass 1: X = x^T x (big arena) + G0 = x^T Wp0 (mm/u slots) ----
        X_ps = big_ps.tile([128, EB, E], F32, tag="big")
        g_sb = [
            consts.tile([128, EB, E], BF16, tag=f"g{hl}", name=f"g{hl}")
            for hl in range(2)
        ]
        g0_slots = [
            mm_ps.tile([128, E], F32, tag="mm", name="g0a"),
            mm_ps.tile([128, E], F32, tag="mm", name="g0b"),
            u_ps.tile([128, E], F32, tag="u", name="g0c"),
            u_ps.tile([128, E], F32, tag="u", name="g0d"),
        ]
        gate_g0 = None
        for np_ in range(NP2):
            xt = xn_sb[np_]
            wpt = wp_sb[(0, np_)]
            # X first (triangle: only columns >= m), then G0 -- gives the wp
            # stream extra slack so its DMA never stalls the PE
            for m in range(EB):
                msl = slice(m * 128, (m + 1) * 128)
                for j in range(2):
                    nc.tensor.matmul(
                        X_ps[:, m, m * 128 :],
                        xt[:, j, msl],
                        xt[:, j, m * 128 :],
                        start=np_ == 0 and j == 0,
                        stop=np_ == NP2 - 1 and j == 1,
                    )
            for m in range(EB):
                msl = slice(m * 128, (m + 1) * 128)
                for j in range(2):
                    g_bi = nc.tensor.matmul(
                        g0_slots[m][:],
                        xt[:, j, msl],
                        wpt[:, j, :],
                        start=np_ == 0 and j == 0,
                        stop=np_ == NP2 - 1 and j == 1,
                    )
                if np_ == 1 and m == 0:
                    gate_g0 = g_bi.ins
            if np_ == 1:
                for bi in wp1_bis:
                    tile.add_dep_helper(bi.ins, gate_g0, reason="delay wp1")
        # g casts first (they gate pass-2 psum reuse), X casts after
        nc.vector.tensor_copy(g_sb[0][:, 0, :], g0_slots[0][:])
        nc.scalar.activation(g_sb[0][:, 1, :], g0_slots[1][:], COPY)
        nc.vector.tensor_copy(g_sb[0][:, 2, :], g0_slots[2][:])
        nc.scalar.activation(g_sb[0][:, 3, :], g0_slots[3][:], COPY)
        X_sb = consts.tile([128, EB, E], BF16, tag="X")
        nc.vector.tensor_copy(X_sb[:, 0, :], X_ps[:, 0, :])
        nc.scalar.activation(X_sb[:, 1, 128:], X_ps[:, 1, 128:], COPY)
        nc.vector.tensor_copy(X_sb[:, 2, 256:], X_ps[:, 2, 256:])
        nc.scalar.activation(X_sb[:, 3, 384:], X_ps[:, 3, 384:], COPY)

        # ---- pass 2: G1 = x^T Wp1 ----
        g1_slots = [
            mm_ps.tile([128, E], F32, tag="mm", name="g1a"),
            mm_ps.tile([128, E], F32, tag="mm", name="g1b"),
            u_ps.tile([128, E], F32, tag="u", name="g1c"),
            u_ps.tile([128, E], F32, tag="u", name="g1d"),
        ]
        gate_g1 = None
        for np_ in range(NP2):
            xt = xn_sb[np_]
            wpt = wp_sb[(1, np_)]
            for m in range(EB):
                msl = slice(m * 128, (m + 1) * 128)
                for j in range(2):
                    g_bi = nc.tensor.matmul(
                        g1_slots[m][:],
                        xt[:, j, msl],
                        wpt[:, j, :],
                        start=np_ == 0 and j == 0,
                        stop=np_ == NP2 - 1 and j == 1,
                    )
                if np_ == 1 and m == 0:
                    gate_g1 = g_bi.ins
            if np_ == 2:
                # X is symmetric: fill the lower-triangle blocks with
                # SBUF->SBUF DMA transposes of the upper blocks (no PE cost)
                engs = [nc.sync, nc.scalar]
                k = 0
                for mt in range(1, EB):
                    for ct in range(mt):
                        engs[k % 2].dma_start_transpose(
                            out=X_sb[:, mt, ct * 128 : (ct + 1) * 128],
                            in_=X_sb[:, ct, mt * 128 : (mt + 1) * 128],
                        )
                        k += 1
        nc.vector.tensor_copy(g_sb[1][:, 0, :], g1_slots[0][:])
        nc.scalar.activation(g_sb[1][:, 1, :], g1_slots[1][:], COPY)
        nc.vector.tensor_copy(g_sb[1][:, 2, :], g1_slots[2][:])
        nc.scalar.activation(g_sb[1][:, 3, :], g1_slots[3][:], COPY)

        gate_hist = [gate_g0, gate_g1]  # per-head early gates
        pending = {"srow": None}

        def pump_s():
            if pending["srow"] is not None:
                A_sb, R_list = pending["srow"]
                pending["srow"] = None
                s_ps = mm_ps.tile([128, EB, 2], F32, tag="mm")
                for eb in range(EB):
                    esl = slice(eb * 128, (eb + 1) * 128)
                    for fc in range(EB):
                        nc.tensor.matmul(
                            s_ps[:, eb, :],
                            A_sb[:, fc, esl],
                            ones_sb[:],
                            start=fc == 0,
                            stop=fc == EB - 1,
                        )
                r_sb = r_pool.tile([128, EB, 2], F32, tag="r")
                nc.vector.reciprocal(out=r_sb[:], in_=s_ps[:])
                R_list.append(r_sb)

        for u in range(2):
            A_tiles, R_tiles = [], []
            for hl in range(2):
                # --- weights for head (u, hl), prefetch-gated ---
                wv_sb = wqkv_pool.tile([128, EB, E], BF16, tag="wv")
                wv_bi = nc.gpsimd.dma_start(out=wv_sb[:], in_=wv_d.ap()[u, hl])
                wq_sb = wqkv_pool.tile([128, EB, E], BF16, tag="wq")
                wq_bi = nc.gpsimd.dma_start(out=wq_sb[:], in_=wq_d.ap()[u, hl])
                wk_sb = wqkv_pool.tile([128, EB, E], BF16, tag="wk")
                wk_bi = nc.sync.dma_start(out=wk_sb[:], in_=wk_d.ap()[u, hl])
                gate = gate_hist[-2]  # two head-phases back
                for bi in (wv_bi, wq_bi, wk_bi):
                    tile.add_dep_helper(bi.ins, gate, reason="delay prefetch")

                P_sb = P_sbs[hl]

                # --- P_h = Wv_h^T @ G_hl + bv (x) swp + bp/2 ---
                def emit_p_group(fb, use_act):
                    fsl = slice(fb * 128, (fb + 1) * 128)
                    p_ps = (u_ps if fb % 2 == 0 else mm_ps).tile(
                        [128, E], F32, tag="u" if fb % 2 == 0 else "mm",
                        name=f"pp{fb}",
                    )
                    first = None
                    for ec in range(EB):
                        bi = nc.tensor.matmul(
                            p_ps[:],
                            wv_sb[:, ec, fsl],
                            g_sb[hl][:, ec, :],
                            start=ec == 0,
                            stop=False,
                        )
                        first = first or bi
                    nc.tensor.matmul(
                        p_ps[:],
                        pb2_sb[0:2, u, hl, 0, fsl],
                        pb2_sb[0:2, u, hl, 1, :],
                        start=False,
                        stop=True,
                    )
                    if use_act:
                        nc.scalar.activation(P_sb[:, fb, :], p_ps[:], COPY)
                    else:
                        nc.vector.tensor_copy(P_sb[:, fb, :], p_ps[:])
                    return first

                def emit_t1(vector_only):
                    T1_ps = big_ps.tile([128, EB, E], F32, tag="big")
                    for m in range(EB):
                        msl = slice(m * 128, (m + 1) * 128)
                        for ec in range(EB):
                            nc.tensor.matmul(
                                T1_ps[:, m, :],
                                X_sb[:, ec, msl],
                                wq_sb[:, ec, :],
                                start=ec == 0,
                                stop=ec == EB - 1,
                            )
                    T1_sb = t1_pool.tile([128, EB, E], BF16, tag="t1")
                    for m in range(EB):
                        if vector_only or m % 2 == 0:
                            nc.vector.tensor_copy(T1_sb[:, m, :], T1_ps[:, m, :])
                        else:
                            nc.scalar.activation(T1_sb[:, m, :], T1_ps[:, m, :], COPY)
                    return T1_sb

                if u == 0 and hl == 0:
                    # first head: nothing to cover before T1, and P's psum
                    # slots want the G1 casts done -- T1's matmuls cover that
                    T1_sb = emit_t1(False)
                    p_first = emit_p_group(0, False)
                    emit_p_group(1, True)
                    emit_p_group(2, True)
                    emit_p_group(3, False)
                elif hl == 0:
                    # first half of P covers the previous head's exp wait
                    p_first = emit_p_group(0, False)
                    emit_p_group(1, True)
                    T1_sb = emit_t1(False)
                    # second half of P covers the T1 casts
                    emit_p_group(2, True)
                    emit_p_group(3, False)
                else:
                    # second head: T1's arena waits for the first head's exp
                    # to drain, so put three P groups ahead of it; all casts
                    # go to Vector so Scalar starts exp with zero queue delay
                    p_first = emit_p_group(0, False)
                    emit_p_group(1, False)
                    emit_p_group(2, False)
                    T1_sb = emit_t1(True)
                    emit_p_group(3, False)
                gate_hist.append(p_first.ins)
                pump_s()

                # --- attT = Wk_h^T @ T1 + rk1 rank-2 update; each block's
                # exp (softmax numerator) chases its accumulation stop ---
                attT_ps = big_ps.tile([128, EB, E], F32, tag="big")
                A_sb = a_pool.tile([128, EB, E], BF16, tag="a")
                for fb in range(EB):
                    fsl = slice(fb * 128, (fb + 1) * 128)
                    for ec in range(EB):
                        nc.tensor.matmul(
                            attT_ps[:, fb, :],
                            wk_sb[:, ec, fsl],
                            T1_sb[:, ec, :],
                            start=ec == 0,
                            stop=False,
                        )
                    nc.tensor.matmul(
                        attT_ps[:, fb, :],
                        rk1_sb[0:2, u, hl, 0, fsl],
                        rk1_sb[0:2, u, hl, 1, :],
                        start=False,
                        stop=True,
                    )
                    nc.scalar.activation(
                        out=A_sb[:, fb, :],
                        in_=attT_ps[:, fb, :],
                        func=EXP,
                        scale=SCALE,
                    )
                A_tiles.append(A_sb)
                pending["srow"] = (A_sb, R_tiles)

            # --- U_h = A_h^T @ P_h ; out = U0*r0 + U1*r1 ---
            out_tiles = [
                o_pool.tile([128, E], BF16, tag="o", name=f"ot{u}_{i}")
                for i in range(EB)
            ]
            for hl in range(2):
                if hl == 1:
                    pump_s()  # s of this unit's second head
                for eb in range(EB):
                    esl = slice(eb * 128, (eb + 1) * 128)
                    u_tile = (u_ps if eb % 2 == 0 else mm_ps).tile(
                        [128, E], F32, tag="u" if eb % 2 == 0 else "mm",
                        name=f"ut{hl}_{eb}",
                    )
                    for fc in range(EB):
                        nc.tensor.matmul(
                            u_tile[:],
                            A_tiles[hl][:, fc, esl],
                            P_sbs[hl][:, fc, :],
                            start=fc == 0,
                            stop=fc == EB - 1,
                        )
                    if hl == 0:
                        # per-partition scaled copy on ACT: frees Vector and
                        # releases the U psum WAR from a second engine
                        nc.scalar.activation(
                            out_tiles[eb][:],
                            u_tile[:],
                            COPY,
                            scale=R_tiles[0][:, eb, 0:1],
                        )
                    else:
                        nc.vector.scalar_tensor_tensor(
                            out_tiles[eb][:],
                            u_tile[:],
                            R_tiles[1][:, eb, 0:1],
                            out_tiles[eb][:],
                            op0=mybir.AluOpType.mult,
                            op1=mybir.AluOpType.add,
                        )
                        (nc.sync if eb % 2 == 0 else nc.scalar).dma_start(
                            out=out_d.ap()[u, eb * 128 : (eb + 1) * 128, :],
                            in_=out_tiles[eb][:],
                        )

    nc.compile()
    return nc


def _get_nc():
    global _CACHED_NC
    if _CACHED_NC is None:
        _CACHED_NC = build_nc()
    return _CACHED_NC


def make_in_maps(x, Wq, bq, Wk, bk, Wv, bv, Wp, bp):
    x = np.asarray(x, np.float32)
    Wq, Wk, Wv, Wp = (np.asarray(a, np.float32) for a in (Wq, Wk, Wv, Wp))
    bq, bk, bv, bp = (np.asarray(a, np.float32) for a in (bq, bk, bv, bp))

    def b16(a):
        return np.ascontiguousarray(a.astype(NP_BF16))

    wp_arr = b16(np.stack([Wp[:N], Wp[N:]]))
    swp = np.stack([Wp[:N].sum(0), Wp[N:].sum(0)])  # (2, E)
    in_maps = []
    for c in range(8):
        b = c // 2
        rs = [2 * (c % 2), 2 * (c % 2) + 1]
        heads = [[2 * r + hl for hl in range(2)] for r in rs]
        xs = x[b].sum(0)

        def tile_w(Wm, h):
            # (E, E) -> [p, t, e] with row t*128+p on partition p
            return Wm[:, h::H].reshape(EB, 128, E).transpose(1, 0, 2)

        wq_arr = b16(np.stack([[tile_w(Wq, h) for h in hu] for hu in heads]))
        wk_arr = b16(np.stack([[tile_w(Wk, h) for h in hu] for hu in heads]))
        wv_arr = b16(np.stack([[tile_w(Wv, h) for h in hu] for hu in heads]))
        # pb2[part, u, hl, role, :]: rank-2 P bias update:
        #   part 0: bv (lhsT) / swp_hl (rhs);  part 1: ones (lhsT) / bp/2
        pb2 = np.empty((2, 2, 2, 2, E), np.float32)
        # rk1[part, u, hl, role, :]: merged rank-2 bias update of attT:
        #   part 0: (Wk^T xs) (x) bq ; part 1: bk (x) (Wq^T xs + N bq)
        rk1 = np.empty((2, 2, 2, 2, E), np.float32)
        for iu, hu in enumerate(heads):
            for ihl, h in enumerate(hu):
                pb2[0, iu, ihl, 0] = bv[h::H]
                pb2[0, iu, ihl, 1] = swp[ihl]
                pb2[1, iu, ihl, 0] = 1.0
                pb2[1, iu, ihl, 1] = 0.5 * bp
                rk1[0, iu, ihl, 0] = Wk[:, h::H].T @ xs
                rk1[0, iu, ihl, 1] = bq[h::H]
                rk1[1, iu, ihl, 0] = bk[h::H]
                rk1[1, iu, ihl, 1] = Wq[:, h::H].T @ xs + np.float32(N) * bq[h::H]
        in_maps.append(
            {
                "xn": b16(x[b]),
                "wp": wp_arr,
                "wq": wq_arr,
                "wk": wk_arr,
                "wv": wv_arr,
                "pb2": b16(pb2),
                "rk1": b16(rk1),
            }
        )
    return in_maps


def assemble_out(results):
    out = np.empty((B, N, E), np.float32)
    for c in range(8):
        b = c // 2
        for ui in range(2):
            r = 2 * (c % 2) + ui
            out[b, r::4, :] = np.asarray(results[c]["out"][ui], np.float32)
    return out


def run(inputs, trace=False, **spmd_kwargs):
    """Full pipeline; returns (output, BassKernelResults)."""
    nc = _get_nc()
    in_maps = make_in_maps(**inputs)
    res = run_bass_kernel_spmd(
        nc, in_maps, core_ids=list(range(8)), trace=trace, **spmd_kwargs
    )
    return assemble_out(res.results), res


def kernel(**inputs):
    out, _ = run(inputs)
    return out
